# revision 9
# baseline (speedup 1.0000x reference)
"""CTC loss (keras ctc_batch_cost semantics) on 8 Trainium2 NeuronCores.

Strategy (pure data parallelism, batch sharded 8 ways; 512 batches/core):
  - Host pre-transposes y_pred to [C, B, T] (bf16) so the device receives
    c-on-partitions tiles, and builds per-batch one-hot matrices
    W[c, b, s] = (ext[b, s] == c) in bf16.
  - Stage 1 (PE): per batch b, qx_b[t, s] = ypT_b[c, t].T @ W_b[c, s] — a
    96x128 stationary / 96x33 moving matmul. This IS the extended-label
    gather (one-hot contraction), replacing the slow GPSIMD ap_gather
    (~250us) with ~35us of PE time. Outputs packed 15 batches per PSUM
    bank, drained by Act to bf16 qx[t-part, (b, s)] tiles.
  - Stage 2 (PE): 33 transposes per 128-batch group regroup [t, b] ->
    [b, t] per s (bf16 transposes, 8 s-slices packed per PSUM bank), Act
    drains to the DP input qe[b-part, (g, t, s)] bf16 with bias EPS.
  - DP (DVE + Pool): probability-space forward recurrence in bf16
    (2x DVE mode), full-width [128, 4*33] ops; the allow2-mask multiply
    runs on Pool (it's otherwise idle); shared-scale rescale (one scale
    per partition row, tracked via reciprocals) every R=8 steps.
  - Pools are >=2-buffered so iteration n+1's DMA/matmul/transpose work
    overlaps iteration n's DP across engines.

Self-contained: hardcodes shapes from the problem spec.
"""

import numpy as np

# Problem dims (hardcoded per spec nn_CTCLayer_4518305595673)
B, T, C, L = 4096, 128, 96, 16
NCORES = 8
BC = B // NCORES            # 512 batches per core
S = 2 * L + 1               # 33 extended label positions
G4 = BC // 128              # 4 partition groups
BLANK = C - 1               # 95
EPS = 1e-7
R = 8                       # rescale every R time steps (R=16 underflows)
GB = 64                     # batches per DMA chunk of ypt
POOL_MASK_K = 4             # of every 4 steps, this many run mask-mult on Pool
PK1 = 15                    # stage-1 batches packed per PSUM bank (15*33*4B)
PK2 = 8                     # stage-2 s-slices packed per PSUM bank (8*128*2B)

_CACHE = {}


def _build_program(bc=BC, t_len=T, c_dim=C, l_len=L, r_period=R, gb=GB,
                   pool_mask_k=POOL_MASK_K, repeat=1,
                   ph_dma=True, ph_mm=True, ph_trans=True,
                   ph_copy=True, ph_dp=True, dp_dtype="bf16"):
    """Build + compile the per-core Bass program.

    ph_* flags disable pipeline phases for microbenchmarking. Coherent
    chains only: mm needs dma; trans needs mm; copy needs trans. dp can
    run standalone (reads a constant-filled qe).
    """
    import concourse.bacc as bacc
    import concourse.tile as tile
    from concourse import masks, mybir
    from contextlib import ExitStack

    s_len = 2 * l_len + 1
    sg = s_len + 2              # per-group alpha cols: 2 pad + s_len
    g4 = bc // 128
    nbg = bc // gb              # DMA chunks per core (8)
    jpt = 128 // gb             # chunks per 128-batch group (2)
    resc_ts = sorted(set([t for t in range(1, t_len) if t % r_period == 0]
                         + [t_len - 1]))
    nsl = len(resc_ts)

    f32 = mybir.dt.float32
    bf16 = mybir.dt.bfloat16
    dpt = bf16 if dp_dtype == "bf16" else f32

    Alu = mybir.AluOpType
    Act = mybir.ActivationFunctionType
    Ax = mybir.AxisListType

    nc = bacc.Bacc("TRN2", target_bir_lowering=False, debug=False,
                   num_devices=NCORES)
    ypt = nc.dram_tensor("ypt", [c_dim, bc * t_len], bf16,
                         kind="ExternalInput")
    woh = nc.dram_tensor("woh", [c_dim, bc * s_len], bf16,
                         kind="ExternalInput")
    msk = nc.dram_tensor("mask", [128, g4 * s_len], bf16,
                         kind="ExternalInput")
    loss = nc.dram_tensor("loss", [bc, 1], f32, kind="ExternalOutput")

    with tile.TileContext(nc) as tc, ExitStack() as ctx:
        const_pool = ctx.enter_context(tc.tile_pool(name="const", bufs=1))
        load_pool = ctx.enter_context(tc.tile_pool(name="load", bufs=2))
        qx_pool = ctx.enter_context(tc.tile_pool(name="qx", bufs=2))
        psum_pool = ctx.enter_context(
            tc.tile_pool(name="ps", bufs=3, space="PSUM"))
        qe_pool = ctx.enter_context(tc.tile_pool(name="qe", bufs=2))
        dp_pool = ctx.enter_context(tc.tile_pool(name="dp", bufs=2))

        ident = const_pool.tile([128, 128], bf16)
        masks.make_identity(nc, ident[:])
        w_sb = const_pool.tile([c_dim, bc * s_len], bf16)
        nc.sync.dma_start(w_sb[:], woh.ap())
        mask_sb = const_pool.tile([128, g4 * s_len], bf16)
        nc.sync.dma_start(mask_sb[:], msk.ap())
        mv = mask_sb[:].rearrange("p (g s) -> p g s", g=g4)

        qe_const = None
        if not ph_copy:
            # DP-only benchmarking: qe becomes a one-time constant tile so
            # the DP has stable, finite inputs.
            qe_const = const_pool.tile([128, g4 * t_len * s_len], dpt,
                                       tag="qec")
            nc.vector.memset(qe_const[:], 0.5)

        def body():
            if ph_copy:
                qe = qe_pool.tile([128, g4 * t_len * s_len], dpt, tag="qe")
            else:
                qe = qe_const
            qev = qe[:].rearrange("p (g t s) -> p g t s", g=g4, t=t_len)
            # (s, t)-ordered view for the packed stage-2 PSUM->SBUF copies
            qev_st = qe[:].rearrange("p (g t s) -> p g s t", g=g4, t=t_len)

            # ---- stage 1: DMA + one-hot matmul "gather" ----
            for bt in range(g4):
                if not ph_dma:
                    break
                qx = qx_pool.tile([128, 128 * s_len], bf16, tag="qx")
                for jj in range(jpt):
                    j = bt * jpt + jj
                    l_tile = load_pool.tile([c_dim, gb * t_len], bf16,
                                            tag="ld")
                    nc.sync.dma_start(
                        l_tile[:],
                        ypt.ap()[:, gb * t_len * j:gb * t_len * (j + 1)])
                    if not ph_mm:
                        continue
                    bdone = 0
                    while bdone < gb:
                        nb = min(PK1, gb - bdone)
                        pt1 = psum_pool.tile([128, PK1 * s_len], f32,
                                             tag="pt1")
                        for k in range(nb):
                            bi = bdone + k          # batch within chunk
                            bg = j * gb + bi        # batch within core
                            nc.tensor.matmul(
                                pt1[:, s_len * k:s_len * (k + 1)],
                                l_tile[:, t_len * bi:t_len * (bi + 1)],
                                w_sb[:, s_len * bg:s_len * (bg + 1)],
                            )
                        qx0 = (jj * gb + bdone) * s_len
                        nc.scalar.activation(
                            qx[:, qx0:qx0 + nb * s_len],
                            pt1[:, :nb * s_len], Act.Copy)
                        bdone += nb
                if not (ph_mm and ph_trans):
                    continue
                # ---- stage 2: regroup [t, b] -> [b, t] per s ----
                gvx = qx[:].rearrange("p (i s) -> p i s", i=128)
                for s0 in range(0, s_len, PK2):
                    ns = min(PK2, s_len - s0)
                    pt2 = psum_pool.tile([128, 128 * PK2], bf16, tag="pt2")
                    for k in range(ns):
                        nc.tensor.transpose(pt2[:, 128 * k:128 * (k + 1)],
                                            gvx[:, :, s0 + k], ident[:])
                    if not ph_copy:
                        continue
                    src = pt2[:].rearrange("p (s t) -> p s t", s=PK2)[:, :ns, :]
                    nc.scalar.activation(qev_st[:, bt, s0:s0 + ns, :], src,
                                         Act.Copy, bias=EPS)

            if not ph_dp:
                loss_sb0 = dp_pool.tile([128, g4], f32, tag="loss_sb")
                nc.vector.memset(loss_sb0[:], 0.0)
                nc.sync.dma_start(
                    loss.ap().rearrange("(g p) one -> p (g one)", p=128),
                    loss_sb0[:])
                return

            # ---- DP phase (prob space, shared-scale rescale) ----
            alpha_a = dp_pool.tile([128, g4 * sg], dpt, tag="alpha_a")
            alpha_b = dp_pool.tile([128, g4 * sg], dpt, tag="alpha_b")
            a_tiles = [alpha_a, alpha_b]
            for a in a_tiles:
                nc.vector.memset(a[:], 0.0)
            av = [a[:].rearrange("p (g s) -> p g s", g=g4) for a in a_tiles]

            u_t = dp_pool.tile([128, g4 * s_len], dpt, tag="u_t")
            v_t = dp_pool.tile([128, g4 * s_len], dpt, tag="v_t")
            uv = u_t[:].rearrange("p (g s) -> p g s", g=g4)
            vv = v_t[:].rearrange("p (g s) -> p g s", g=g4)

            rec = dp_pool.tile([128, nsl], f32, tag="rec")
            mx = dp_pool.tile([128, 1], f32, tag="mx")

            # t = 0 init: alpha[s=0,1] = q'[0, s], rest 0
            nc.vector.tensor_copy(av[0][:, :, 2:4], qev[:, :, 0, 0:2])

            cur = 0
            for t in range(1, t_len):
                prev, nxt = av[cur], av[1 - cur]
                nc.vector.tensor_tensor(uv[:, :, :], prev[:, :, 2:2 + s_len],
                                        prev[:, :, 1:1 + s_len], op=Alu.add)
                veng = (nc.gpsimd if (t % 4) < pool_mask_k else nc.vector)
                veng.tensor_tensor(vv[:, :, :], prev[:, :, 0:s_len],
                                   mv[:, :, :], op=Alu.mult)
                nc.vector.tensor_tensor(uv[:, :, :], uv[:, :, :], vv[:, :, :],
                                        op=Alu.add)
                nc.vector.tensor_tensor(nxt[:, :, 2:2 + s_len], uv[:, :, :],
                                        qev[:, :, t, :], op=Alu.mult)
                if t in resc_ts:
                    slot = resc_ts.index(t)
                    nc.vector.tensor_reduce(mx[:], a_tiles[1 - cur][:, :],
                                            axis=Ax.X, op=Alu.max)
                    nc.vector.reciprocal(rec[:, slot:slot + 1], mx[:])
                    nc.vector.tensor_scalar(
                        a_tiles[1 - cur][:, :], a_tiles[1 - cur][:, :],
                        rec[:, slot:slot + 1], None, op0=Alu.mult)
                cur = 1 - cur

            # ---- epilogue ----
            lg = dp_pool.tile([128, nsl], f32, tag="lg")
            nc.scalar.activation(lg[:], rec[:], Act.Ln)
            lsum = dp_pool.tile([128, 1], f32, tag="lsum")
            nc.vector.tensor_reduce(lsum[:], lg[:], axis=Ax.X, op=Alu.add)
            fin = av[cur]
            tail = dp_pool.tile([128, g4], f32, tag="tail")
            nc.vector.tensor_tensor(tail[:], fin[:, :, sg - 2],
                                    fin[:, :, sg - 1], op=Alu.add)
            ltail = dp_pool.tile([128, g4], f32, tag="ltail")
            nc.scalar.activation(ltail[:], tail[:], Act.Ln)
            # ll = sum_t ln(scale_t) + ln(tail) = -lsum + ltail
            # (lsum = sum ln(rec), rec = 1/scale), so loss = lsum - ltail.
            loss_sb = dp_pool.tile([128, g4], f32, tag="loss_sb")
            nc.vector.tensor_scalar(loss_sb[:], ltail[:], -1.0, lsum[:, 0:1],
                                    op0=Alu.mult, op1=Alu.add)
            nc.sync.dma_start(
                loss.ap().rearrange("(g p) one -> p (g one)", p=128),
                loss_sb[:])

        for _rep in range(repeat):
            body()

    nc.compile()
    return nc


def _host_prep(y_true, y_pred, bc=BC, gb=GB, s_len=S):
    """Shard + build transposed-prob / one-hot / mask tensors."""
    import ml_dtypes

    y_true = np.asarray(y_true).astype(np.int64)
    y_pred = np.asarray(y_pred).astype(np.float32)
    ncores = y_pred.shape[0] // bc
    g4 = bc // 128
    ext = np.full((y_true.shape[0], s_len), BLANK, dtype=np.int64)
    ext[:, 1::2] = y_true
    mask_full = np.zeros((ext.shape[0], s_len), dtype=np.float32)
    mask_full[:, 2:] = ((ext[:, 2:] != ext[:, :-2])
                        & (ext[:, 2:] != BLANK)).astype(np.float32)

    # one-hot W[c, b, s] = (ext[b, s] == c), bf16
    woh_full = (ext[None, :, :] == np.arange(C)[:, None, None])

    in_maps = []
    for cid in range(ncores):
        b0 = cid * bc
        ypt_c = np.ascontiguousarray(
            y_pred[b0:b0 + bc].transpose(2, 0, 1).reshape(C, bc * T)
        ).astype(ml_dtypes.bfloat16)
        woh_c = np.ascontiguousarray(
            woh_full[:, b0:b0 + bc, :].reshape(C, bc * s_len)
        ).astype(ml_dtypes.bfloat16)
        m = mask_full[b0:b0 + bc].reshape(g4, 128, s_len).transpose(1, 0, 2)
        mask_c = np.ascontiguousarray(m.reshape(128, g4 * s_len)).astype(
            ml_dtypes.bfloat16)
        in_maps.append({"ypt": ypt_c, "woh": woh_c, "mask": mask_c})
    return in_maps


def get_program(repeat=1):
    key = ("nc", repeat)
    if key not in _CACHE:
        _CACHE[key] = _build_program(repeat=repeat)
    return _CACHE[key]


def kernel(y_true, y_pred):
    from concourse import bass_utils
    nc = get_program()
    in_maps = _host_prep(y_true, y_pred)
    res = bass_utils.run_bass_kernel_spmd(nc, in_maps,
                                          core_ids=list(range(NCORES)))
    out = np.concatenate([res.results[c]["loss"] for c in range(NCORES)],
                         axis=0)
    return out.astype(np.float32)
